# revision 25
# baseline (speedup 1.0000x reference)
"""Multi-head attention (B=2, S=4096, D=768, H=12) on 8 Trainium2 cores.

Sharding: (batch, head-group) -> core.  Core c handles batch c//4 and heads
3*(c%4) .. 3*(c%4)+2.  Q/K/V projections are computed per-core on the head
slice of the weights; the output projection is computed as a partial product
over the core's 192 combined-head dims and the 4 partials per batch are summed
on the host (the "all-reduce").

Device-side layout choices:
  - Host pre-transposes inputs to X^T [768, S] and pre-casts everything to
    bf16, so no on-device transposes of the big inputs are needed.
  - q^T, k^T are produced directly by the projection matmuls in [dk, S]
    layout (d on partitions) and replicated into both 64-partition halves so
    the K=64 scores matmuls can be row-packed two-at-a-time with
    tile_position (0,0)/(64,0).
  - scores are computed transposed: s^T[kpos, q] = k^T.T @ q^T, so softmax's
    exp is a single layout-agnostic ACT pass and the probs land exactly in
    the lhsT layout the attn@v matmul wants.  No max-subtraction: scores are
    ~N(0,1) here, exp is safe in fp32.
  - v carries an extra ones-column, so attn^T row 64 accumulates the softmax
    denominator for free.
  - normalization happens via a small double-transpose epilogue on [65, 512]
    tiles and is folded into psum->sbuf copies; the 1/8 scale is folded into
    Wq; biases bq/bk are folded into the projection copies; bv and bo are
    applied on the host (bv contributes a constant row through softmax).
"""

import os
import sys

import numpy as np

for _p in ("/opt/trn_rl_repo", "/root/.axon_site/_ro/trn_rl_repo"):
    if _p not in sys.path and os.path.isdir(_p):
        sys.path.append(_p)

import concourse.bass as bass
import concourse.mybir as mybir
import concourse.tile as tile
from concourse.bass_utils import run_bass_kernel_spmd
from concourse.masks import make_identity

try:
    from ml_dtypes import bfloat16 as _bf16np
except ImportError:  # pragma: no cover
    _bf16np = np.dtype("bfloat16").type

F32 = mybir.dt.float32
BF16 = mybir.dt.bfloat16
I16 = mybir.dt.int16

D_MODEL = 768
N_HEADS_CORE = 3  # heads per core
DH = 192  # N_HEADS_CORE * 64
KCH = D_MODEL // 128  # contraction chunks for projections

# exp split: ScalarE computes true exp for most kpos-tile pairs; VectorE
# covers the pairs listed in DVE_PAIRS (of each 16-pair (h, qc) unit) with
# the Schraudolph bit-trick: bf16 bits ~= round(A*s + B), evaluated as an
# int16 tensor_scalar into a dedicated int16 tile (single writer per tile,
# so Tile never serializes the two engines).
DVE_PAIRS = (1, 3, 6, 9, 11, 14)
SCH_A = 128.0 / float(np.log(2.0))
SCH_B = 16248.6


def split_multi_waits(nc, max_waits=1):
    """This container's walrus rejects >1 semaphore wait per instruction
    (setupSyncWait).  Move excess waits onto same-engine NoOps just before
    the offending instruction."""
    n = 0
    for f in nc.m.functions:
        for bb in f.blocks:
            out = []
            for inst in bb.instructions:
                si = inst.sync_info
                if si is not None and si.on_wait and len(si.on_wait) > max_waits:
                    waits = list(si.on_wait)
                    for j, w in enumerate(waits[:-max_waits]):
                        out.append(
                            mybir.InstNoOp(
                                name=f"{inst.name}-wsplit{j}",
                                engine=inst.engine,
                                ins=[],
                                outs=[],
                                sync_info=mybir.SyncInfo(on_wait=[w], on_update=[]),
                            )
                        )
                    si.on_wait = waits[-max_waits:]
                    n += 1
                out.append(inst)
            bb.instructions = out
    return n


def build_nc(S, split=True):
    assert S % 512 == 0
    NQ = S // 512  # query chunks / projection chunks
    NT = S // 128  # kpos tiles
    nc = bass.Bass()

    xqT = nc.declare_dram_parameter("xqT", [D_MODEL, S], BF16, isOutput=False)
    xkT = nc.declare_dram_parameter("xkT", [D_MODEL, S], BF16, isOutput=False)
    xvT = nc.declare_dram_parameter("xvT", [D_MODEL, S], BF16, isOutput=False)
    wqT = nc.declare_dram_parameter("wqT", [D_MODEL, DH], BF16, isOutput=False)
    wkT = nc.declare_dram_parameter("wkT", [D_MODEL, DH], BF16, isOutput=False)
    wvT = nc.declare_dram_parameter("wvT", [D_MODEL, DH], BF16, isOutput=False)
    bq = nc.declare_dram_parameter("bq", [DH, 1], F32, isOutput=False)
    bk = nc.declare_dram_parameter("bk", [DH, 1], F32, isOutput=False)
    comb_out = nc.declare_dram_parameter("comb", [S, DH], BF16, isOutput=True)

    with tile.TileContext(nc) as tc:
        with (
            tc.tile_pool(name="consts", bufs=1) as consts,
            tc.tile_pool(name="persist", bufs=1) as persist,
            tc.tile_pool(name="xin", bufs=7) as xin,
            tc.tile_pool(name="probs", bufs=3) as probs_pool,
            tc.tile_pool(name="epi_sb", bufs=3) as epi_sb,
            tc.tile_pool(name="small", bufs=8) as small,
        ):
            # ---- constants ----
            id_f32 = consts.tile([128, 128], F32, tag="id_f32")
            make_identity(nc, id_f32)
            # preload the exp ACT table during the projection head
            warm = consts.tile([1, 2], F32, tag="warm")
            nc.vector.memset(warm, 0.0)
            nc.scalar.activation(
                out=warm, in_=warm, func=mybir.ActivationFunctionType.Exp
            )
            bq_lo = consts.tile([128, 1], F32, tag="bq_lo")
            nc.gpsimd.dma_start(out=bq_lo, in_=bq[0:128, :])
            bq_hi = consts.tile([64, 1], F32, tag="bq_hi")
            nc.gpsimd.dma_start(out=bq_hi, in_=bq[128:DH, :])
            bk_lo = consts.tile([128, 1], F32, tag="bk_lo")
            nc.gpsimd.dma_start(out=bk_lo, in_=bk[0:128, :])
            bk_hi = consts.tile([64, 1], F32, tag="bk_hi")
            nc.gpsimd.dma_start(out=bk_hi, in_=bk[128:DH, :])
            wq_sb = consts.tile([128, KCH, DH], BF16, tag="wq_sb")
            nc.scalar.dma_start(out=wq_sb, in_=wqT.rearrange("(c p) n -> p c n", p=128))
            wk_sb = consts.tile([128, KCH, DH], BF16, tag="wk_sb")
            nc.scalar.dma_start(out=wk_sb, in_=wkT.rearrange("(c p) n -> p c n", p=128))
            wv_sb = consts.tile([128, KCH, DH], BF16, tag="wv_sb")
            nc.scalar.dma_start(out=wv_sb, in_=wvT.rearrange("(c p) n -> p c n", p=128))

            # ---- persistent activations, per 512-col chunk ----
            qTr = [[persist.tile([128, 512], BF16, tag=f"qTr{h}_{c}", name=f"qTr{h}_{c}")
                    for c in range(NQ)] for h in range(3)]
            kTr = [[persist.tile([128, 512], BF16, tag=f"kTr{h}_{c}", name=f"kTr{h}_{c}")
                    for c in range(NQ)] for h in range(3)]
            v_c = [persist.tile([128, 4, 3, 65], BF16, tag=f"v_{c}", name=f"v_{c}")
                   for c in range(NQ)]
            for c in range(NQ):
                nc.vector.memset(v_c[c][:, :, :, 64:65], 1.0)


            with (
                tc.tile_pool(name="ps_big", bufs=2, space="PSUM") as ps_big,
                tc.tile_pool(name="ps_acc", bufs=1, space="PSUM") as ps_acc,
                tc.tile_pool(name="ps_epi1", bufs=1, space="PSUM") as ps_epi1,
                tc.tile_pool(name="ps_proj", bufs=2, space="PSUM") as ps_proj,
            ):

                def qk_halfproj(w_sb, x_t, dst, blo, bhi, c, half):
                    # one 256-col half of a q/k projection chunk, single psum
                    # bank; half indexes x_t (which may hold 2 chunks), dst
                    # columns use half%2 within chunk c
                    hc = bass.ds(half * 256, 256)
                    dc = bass.ds((half % 2) * 256, 256)
                    pst = ps_proj.tile([128, 512], F32, tag="proj", name="pst")
                    ps0 = pst[:, 0:256]
                    ps1 = pst[0:64, 256:512]
                    for kk in range(KCH):
                        nc.tensor.matmul(ps0, w_sb[:, kk, 0:128], x_t[:, kk, hc],
                                         start=(kk == 0), stop=(kk == KCH - 1),
                                         skip_group_check=True)
                    for kk in range(KCH):
                        nc.tensor.matmul(ps1, w_sb[:, kk, 128:DH], x_t[:, kk, hc],
                                         start=(kk == 0), stop=(kk == KCH - 1),
                                         skip_group_check=True)
                    nc.vector.tensor_scalar_add(dst[0][c][0:64, dc], ps0[0:64, :], blo[0:64])
                    nc.sync.dma_start(out=dst[0][c][64:128, dc], in_=dst[0][c][0:64, dc])
                    nc.vector.tensor_scalar_add(dst[1][c][64:128, dc], ps0[64:128, :], blo[64:128])
                    nc.sync.dma_start(out=dst[1][c][0:64, dc], in_=dst[1][c][64:128, dc])
                    nc.vector.tensor_scalar_add(dst[2][c][0:64, dc], ps1[0:64, :], bhi[0:64])
                    nc.sync.dma_start(out=dst[2][c][64:128, dc], in_=dst[2][c][0:64, dc])

                def kv_group_pieces(p):
                    # fine-grained projection pieces for k/v chunk pair
                    # (2p, 2p+1), to be spread across inner-loop units
                    chunks = [c for c in (2 * p, 2 * p + 1) if c < NQ]
                    w = 512 * len(chunks)
                    ncols = bass.ds(chunks[0] * 512, w)
                    st = {}
                    k_eng = nc.gpsimd if p == 0 else nc.sync

                    def k_dma():
                        xk_t = xin.tile([128, KCH, w], BF16, tag="x_t",
                                        name="xk_t")
                        k_eng.dma_start(
                            out=xk_t,
                            in_=xkT.rearrange("(cc p) s -> p cc s",
                                              p=128)[:, :, ncols])
                        st["k"] = xk_t

                    def v_dma():
                        xv_t = xin.tile([128, KCH, w], BF16, tag="x_t",
                                        name="xv_t")
                        nc.scalar.dma_start(
                            out=xv_t,
                            in_=xvT.rearrange("(cc p) s -> p cc s",
                                              p=128)[:, :, ncols])
                        st["v"] = xv_t

                    def k_half(ci, half):
                        def f():
                            qk_halfproj(wk_sb, st["k"], kTr, bk_lo, bk_hi,
                                        chunks[ci], 2 * ci + half)
                        return f

                    def v_chunk(ci):
                        def f():
                            for sub in range(4):
                                vps = ps_proj.tile([128, 512], F32, tag="proj",
                                                   name="vps")
                                for kk in range(KCH):
                                    nc.tensor.matmul(
                                        vps[:, 0:DH],
                                        st["v"][:, kk,
                                                bass.ds(ci * 512 + sub * 128,
                                                        128)],
                                        wv_sb[:, kk, :],
                                        start=(kk == 0), stop=(kk == KCH - 1),
                                        skip_group_check=True,
                                    )
                                nc.vector.tensor_copy(
                                    v_c[chunks[ci]][:, sub, :, 0:64],
                                    vps[:, 0:DH].rearrange("p (h d) -> p h d",
                                                           h=3),
                                )
                        return f

                    out = [k_dma, v_dma, k_half(0, 0), k_half(0, 1)]
                    if len(chunks) > 1:
                        out += [k_half(1, 0), k_half(1, 1)]
                    out.append(v_chunk(0))
                    if len(chunks) > 1:
                        out.append(v_chunk(1))
                    return out

                def qproj_pieces(c):
                    st = {}

                    def p0():
                        xq_t = xin.tile([128, KCH, 512], BF16, tag="x_t",
                                        name="xq_t")
                        nc.sync.dma_start(
                            out=xq_t,
                            in_=xqT.rearrange("(cc p) s -> p cc s",
                                              p=128)[:, :, bass.ts(c, 512)])
                        st["q"] = xq_t
                        qk_halfproj(wq_sb, st["q"], qTr, bq_lo, bq_hi, c, 0)

                    def p1():
                        qk_halfproj(wq_sb, st["q"], qTr, bq_lo, bq_hi, c, 1)

                    return [p0, p1]

                def scores_pair(h, t2, qc):
                    t0, t1 = 2 * t2, 2 * t2 + 1
                    sc = ps_big.tile([128, 1024], F32, tag="big", name="sc")
                    nc.tensor.matmul(
                        sc[:, 0:512],
                        kTr[h][t0 // 4][0:64, bass.ts(t0 % 4, 128)],
                        qTr[h][qc][0:64, :],
                        start=True, stop=True, tile_position=(0, 0),
                    )
                    nc.tensor.matmul(
                        sc[:, 512:1024],
                        kTr[h][t1 // 4][64:128, bass.ts(t1 % 4, 128)],
                        qTr[h][qc][64:128, :],
                        start=True, stop=True, tile_position=(64, 0),
                    )
                    return sc

                def epilogue_job(qc, aT, j):
                    def f():
                        st = qc * 4 + j
                        jc = bass.ts(j, 128)
                        epi = ps_epi1.tile([128, 195], F32, tag="epi1", name="epi")
                        comb = epi_sb.tile([128, DH], BF16, tag="comb", name="comb")
                        for h in range(3):
                            nc.tensor.transpose(
                                epi[:, h * 65 : h * 65 + 65],
                                aT[h][:, jc],
                                id_f32[0:65, 0:65],
                            )
                        epi3 = epi.rearrange("p (h x) -> p h x", h=3)
                        rec = small.tile([128, 3], F32, tag="rec", name="rec")
                        nc.vector.reciprocal(rec, epi3[:, :, 64:65])
                        nc.vector.tensor_tensor(
                            out=comb.rearrange("p (h d) -> p h d", h=3),
                            in0=epi3[:, :, 0:64],
                            in1=rec[:, :, None].to_broadcast([128, 3, 64]),
                            op=mybir.AluOpType.mult,
                        )
                        nc.sync.dma_start(
                            out=comb_out[st * 128 : (st + 1) * 128, :],
                            in_=comb,
                        )
                    return f

                NPAIR = NT // 2
                units = [(qc, h, t2) for qc in range(NQ) for h in range(3)
                         for t2 in range(NPAIR)]
                NU = len(units)
                sc_q = {}

                def emit_scores_for(i):
                    qc_, h_, t2_ = units[i]
                    sc_q[i] = scores_pair(h_, t2_, qc_)

                # deadline-scheduled side work: unit index -> [thunks], run at
                # the top of that unit (before its scores emission)
                sched = {}

                def sched_add(i, thunk):
                    sched.setdefault(i, []).append(thunk)

                # k/v projections for chunk pairs (2p, 2p+1), p = 1..:
                # dmas issue right away (xin bufs cover the lookahead);
                # compute pieces land just ahead of the scores/attn that
                # need them
                for p in range(1, (NQ + 1) // 2):
                    pcs = kv_group_pieces(p)
                    base = 4 * (p - 1)
                    offs = (2 * (p - 1), 2 * (p - 1) + 1,  # kdma, vdma
                            base + 1, base + 2,            # k chunk 2p halves
                            base + 3, base + 4,            # k chunk 2p+1 halves
                            base + 3, base + 5)            # v chunks
                    for off, th in zip(offs, pcs):
                        sched_add(off, th)

                # startup: q chunk 0 + k/v chunks 0,1; first scores go as
                # soon as q proj + k chunk 0 are emitted
                g0 = kv_group_pieces(0)
                qp0 = qproj_pieces(0)
                qp0[0]()
                g0[0]()   # k dma
                qp0[1]()
                g0[2]()   # k chunk 0 half 0
                emit_scores_for(0)
                g0[3]()   # k chunk 0 half 1
                emit_scores_for(1)
                g0[1]()   # v dma
                g0[6]()   # v chunk 0
                g0[4]()   # k chunk 1 half 0
                g0[5]()   # k chunk 1 half 1
                g0[7]()   # v chunk 1

                aT = None
                accs = {}
                for i, (qc, h, t2) in enumerate(units):
                    for th in sched.pop(i, ()):
                        th()
                    if h == 0 and t2 == 0:
                        if qc + 1 < NQ:
                            qp = qproj_pieces(qc + 1)
                            qp[0]()
                            sched_add(i + 16, qp[1])
                        aT = [epi_sb.tile([65, 512], F32, tag=f"aT{hh}",
                                          name=f"aT{hh}") for hh in range(3)]
                    if t2 == 0:
                        accs[h] = ps_acc.tile([65, 512], F32, tag="acc",
                                              name="acc")
                    acc = accs[h]
                    sc_cur = sc_q.pop(i)
                    if t2 in DVE_PAIRS:
                        pri = probs_pool.tile([128, 1024], I16, tag="pri")
                        nc.vector.tensor_scalar(
                            out=pri, in0=sc_cur,
                            scalar1=SCH_A, scalar2=SCH_B,
                            op0=mybir.AluOpType.mult,
                            op1=mybir.AluOpType.add,
                        )
                        pr = pri.bitcast(BF16)
                    else:
                        pr = probs_pool.tile([128, 1024], BF16, tag="pr")
                        nc.scalar.activation(
                            out=pr, in_=sc_cur,
                            func=mybir.ActivationFunctionType.Exp,
                        )
                    if i + 2 < NU:
                        emit_scores_for(i + 2)
                    t0, t1 = 2 * t2, 2 * t2 + 1
                    nc.tensor.matmul(
                        acc, v_c[t0 // 4][:, t0 % 4, h, :], pr[:, 0:512],
                        start=(t2 == 0), stop=False,
                        skip_group_check=True,
                    )
                    nc.tensor.matmul(
                        acc, v_c[t1 // 4][:, t1 % 4, h, :], pr[:, 512:1024],
                        start=False, stop=(t2 == NPAIR - 1),
                        skip_group_check=True,
                    )
                    if t2 == NPAIR - 1:
                        nc.scalar.copy(aT[h], acc)
                        if h == 2:
                            for j in range(4):
                                sched_add(i + 2 + 2 * j, epilogue_job(qc, aT, j))
                for i in sorted(sched):
                    for th in sched[i]:
                        th()

    if split:
        split_multi_waits(nc)
    return nc


_NC_CACHE = {}


def _get_nc(S):
    if S not in _NC_CACHE:
        _NC_CACHE[S] = build_nc(S)
    return _NC_CACHE[S]


def shard_inputs(Q, K, V, Wq, bq, Wk, bk, Wv, bv, Wo, bo, S):
    """Build the 8 per-core input maps (numpy, host-side shard+cast)."""
    in_maps = []
    for c in range(8):
        b = c // 4
        r0 = 3 * (c % 4) * 64
        rows = slice(r0, r0 + DH)
        in_maps.append(
            {
                "xqT": np.ascontiguousarray(Q[b].T).astype(_bf16np),
                "xkT": np.ascontiguousarray(K[b].T).astype(_bf16np),
                "xvT": np.ascontiguousarray(V[b].T).astype(_bf16np),
                "wqT": np.ascontiguousarray(Wq[rows].T / 8.0).astype(_bf16np),
                "wkT": np.ascontiguousarray(Wk[rows].T).astype(_bf16np),
                "wvT": np.ascontiguousarray(Wv[rows].T).astype(_bf16np),
                "bq": (bq[rows] / 8.0).reshape(DH, 1).astype(np.float32),
                "bk": bk[rows].reshape(DH, 1).astype(np.float32),
            }
        )
    return in_maps


def gather_output(results, Q, bv, Wo, bo):
    B, S = Q.shape[0], Q.shape[1]
    out = np.zeros((B, S, D_MODEL), np.float32)
    WoT = Wo.T.astype(np.float32)  # [768 in-dims, 768 out]
    for c, r in enumerate(results):
        r0 = 3 * (c % 4) * 64
        out[c // 4] += r["comb"].astype(np.float32) @ WoT[r0 : r0 + DH, :]
    out += (bv.astype(np.float32) @ WoT + bo.astype(np.float32))[None, None, :]
    return out


def kernel(Q, K, V, Wq, bq, Wk, bk, Wv, bv, Wo, bo, **run_kwargs):
    Q, K, V, Wq, bq, Wk, bk, Wv, bv, Wo, bo = (
        np.asarray(a) for a in (Q, K, V, Wq, bq, Wk, bk, Wv, bv, Wo, bo)
    )
    S = Q.shape[1]
    nc = _get_nc(S)
    in_maps = shard_inputs(Q, K, V, Wq, bq, Wk, bk, Wv, bv, Wo, bo, S)
    res = run_bass_kernel_spmd(nc, in_maps, core_ids=list(range(8)), **run_kwargs)
    out = gather_output(res.results, Q, bv, Wo, bo)
    kernel.last_results = res
    return out



# revision 26
# speedup vs baseline: 1.0129x; 1.0129x over previous
"""Multi-head attention (B=2, S=4096, D=768, H=12) on 8 Trainium2 cores.

Sharding: (batch, head-group) -> core.  Core c handles batch c//4 and heads
3*(c%4) .. 3*(c%4)+2.  Q/K/V projections are computed per-core on the head
slice of the weights; the output projection is computed as a partial product
over the core's 192 combined-head dims and the 4 partials per batch are summed
on the host (the "all-reduce").

Device-side layout choices:
  - Host pre-transposes inputs to X^T [768, S] and pre-casts everything to
    bf16, so no on-device transposes of the big inputs are needed.
  - q^T, k^T are produced directly by the projection matmuls in [dk, S]
    layout (d on partitions) and replicated into both 64-partition halves so
    the K=64 scores matmuls can be row-packed two-at-a-time with
    tile_position (0,0)/(64,0).
  - scores are computed transposed: s^T[kpos, q] = k^T.T @ q^T, so softmax's
    exp is a single layout-agnostic ACT pass and the probs land exactly in
    the lhsT layout the attn@v matmul wants.  No max-subtraction: scores are
    ~N(0,1) here, exp is safe in fp32.
  - v carries an extra ones-column, so attn^T row 64 accumulates the softmax
    denominator for free.
  - normalization happens via a small double-transpose epilogue on [65, 512]
    tiles and is folded into psum->sbuf copies; the 1/8 scale is folded into
    Wq; biases bq/bk are folded into the projection copies; bv and bo are
    applied on the host (bv contributes a constant row through softmax).
"""

import os
import sys

import numpy as np

for _p in ("/opt/trn_rl_repo", "/root/.axon_site/_ro/trn_rl_repo"):
    if _p not in sys.path and os.path.isdir(_p):
        sys.path.append(_p)

import concourse.bass as bass
import concourse.mybir as mybir
import concourse.tile as tile
from concourse.bass_utils import run_bass_kernel_spmd
from concourse.masks import make_identity

try:
    from ml_dtypes import bfloat16 as _bf16np
except ImportError:  # pragma: no cover
    _bf16np = np.dtype("bfloat16").type

F32 = mybir.dt.float32
BF16 = mybir.dt.bfloat16
I16 = mybir.dt.int16

D_MODEL = 768
N_HEADS_CORE = 3  # heads per core
DH = 192  # N_HEADS_CORE * 64
KCH = D_MODEL // 128  # contraction chunks for projections

# exp split: ScalarE computes true exp for most kpos-tile pairs; VectorE
# covers the pairs listed in DVE_PAIRS (of each 16-pair (h, qc) unit) with
# the Schraudolph bit-trick: bf16 bits ~= round(A*s + B), evaluated as an
# int16 tensor_scalar into a dedicated int16 tile (single writer per tile,
# so Tile never serializes the two engines).
DVE_PAIRS = (1, 4, 7, 10, 13)
SCH_A = 128.0 / float(np.log(2.0))
SCH_B = 16248.6


def split_multi_waits(nc, max_waits=1):
    """This container's walrus rejects >1 semaphore wait per instruction
    (setupSyncWait).  Move excess waits onto same-engine NoOps just before
    the offending instruction."""
    n = 0
    for f in nc.m.functions:
        for bb in f.blocks:
            out = []
            for inst in bb.instructions:
                si = inst.sync_info
                if si is not None and si.on_wait and len(si.on_wait) > max_waits:
                    waits = list(si.on_wait)
                    for j, w in enumerate(waits[:-max_waits]):
                        out.append(
                            mybir.InstNoOp(
                                name=f"{inst.name}-wsplit{j}",
                                engine=inst.engine,
                                ins=[],
                                outs=[],
                                sync_info=mybir.SyncInfo(on_wait=[w], on_update=[]),
                            )
                        )
                    si.on_wait = waits[-max_waits:]
                    n += 1
                out.append(inst)
            bb.instructions = out
    return n


def build_nc(S, split=True):
    assert S % 512 == 0
    NQ = S // 512  # query chunks / projection chunks
    NT = S // 128  # kpos tiles
    nc = bass.Bass()

    xqT = nc.declare_dram_parameter("xqT", [D_MODEL, S], BF16, isOutput=False)
    xkT = nc.declare_dram_parameter("xkT", [D_MODEL, S], BF16, isOutput=False)
    xvT = nc.declare_dram_parameter("xvT", [D_MODEL, S], BF16, isOutput=False)
    wqT = nc.declare_dram_parameter("wqT", [D_MODEL, DH], BF16, isOutput=False)
    wkT = nc.declare_dram_parameter("wkT", [D_MODEL, DH], BF16, isOutput=False)
    wvT = nc.declare_dram_parameter("wvT", [D_MODEL, DH], BF16, isOutput=False)
    bq = nc.declare_dram_parameter("bq", [DH, 1], F32, isOutput=False)
    bk = nc.declare_dram_parameter("bk", [DH, 1], F32, isOutput=False)
    comb_out = nc.declare_dram_parameter("comb", [S, DH], BF16, isOutput=True)

    with tile.TileContext(nc) as tc:
        with (
            tc.tile_pool(name="consts", bufs=1) as consts,
            tc.tile_pool(name="persist", bufs=1) as persist,
            tc.tile_pool(name="xin", bufs=7) as xin,
            tc.tile_pool(name="probs", bufs=3) as probs_pool,
            tc.tile_pool(name="epi_sb", bufs=3) as epi_sb,
            tc.tile_pool(name="small", bufs=8) as small,
        ):
            # ---- constants ----
            id_f32 = consts.tile([128, 128], F32, tag="id_f32")
            make_identity(nc, id_f32)
            # preload the exp ACT table during the projection head
            warm = consts.tile([1, 2], F32, tag="warm")
            nc.vector.memset(warm, 0.0)
            nc.scalar.activation(
                out=warm, in_=warm, func=mybir.ActivationFunctionType.Exp
            )
            bq_lo = consts.tile([128, 1], F32, tag="bq_lo")
            nc.gpsimd.dma_start(out=bq_lo, in_=bq[0:128, :])
            bq_hi = consts.tile([64, 1], F32, tag="bq_hi")
            nc.gpsimd.dma_start(out=bq_hi, in_=bq[128:DH, :])
            bk_lo = consts.tile([128, 1], F32, tag="bk_lo")
            nc.gpsimd.dma_start(out=bk_lo, in_=bk[0:128, :])
            bk_hi = consts.tile([64, 1], F32, tag="bk_hi")
            nc.gpsimd.dma_start(out=bk_hi, in_=bk[128:DH, :])
            wq_sb = consts.tile([128, KCH, DH], BF16, tag="wq_sb")
            nc.scalar.dma_start(out=wq_sb, in_=wqT.rearrange("(c p) n -> p c n", p=128))
            wk_sb = consts.tile([128, KCH, DH], BF16, tag="wk_sb")
            nc.scalar.dma_start(out=wk_sb, in_=wkT.rearrange("(c p) n -> p c n", p=128))
            wv_sb = consts.tile([128, KCH, DH], BF16, tag="wv_sb")
            nc.scalar.dma_start(out=wv_sb, in_=wvT.rearrange("(c p) n -> p c n", p=128))

            # ---- persistent activations, per 512-col chunk ----
            qTr = [[persist.tile([128, 512], BF16, tag=f"qTr{h}_{c}", name=f"qTr{h}_{c}")
                    for c in range(NQ)] for h in range(3)]
            kTr = [[persist.tile([128, 512], BF16, tag=f"kTr{h}_{c}", name=f"kTr{h}_{c}")
                    for c in range(NQ)] for h in range(3)]
            v_c = [persist.tile([128, 4, 3, 65], BF16, tag=f"v_{c}", name=f"v_{c}")
                   for c in range(NQ)]
            for c in range(NQ):
                nc.vector.memset(v_c[c][:, :, :, 64:65], 1.0)


            with (
                tc.tile_pool(name="ps_big", bufs=2, space="PSUM") as ps_big,
                tc.tile_pool(name="ps_acc", bufs=1, space="PSUM") as ps_acc,
                tc.tile_pool(name="ps_epi1", bufs=1, space="PSUM") as ps_epi1,
                tc.tile_pool(name="ps_proj", bufs=2, space="PSUM") as ps_proj,
            ):

                def qk_halfproj(w_sb, x_t, dst, blo, bhi, c, half):
                    # one 256-col half of a q/k projection chunk, single psum
                    # bank; half indexes x_t (which may hold 2 chunks), dst
                    # columns use half%2 within chunk c
                    hc = bass.ds(half * 256, 256)
                    dc = bass.ds((half % 2) * 256, 256)
                    pst = ps_proj.tile([128, 512], F32, tag="proj", name="pst")
                    ps0 = pst[:, 0:256]
                    ps1 = pst[0:64, 256:512]
                    for kk in range(KCH):
                        nc.tensor.matmul(ps0, w_sb[:, kk, 0:128], x_t[:, kk, hc],
                                         start=(kk == 0), stop=(kk == KCH - 1),
                                         skip_group_check=True)
                    for kk in range(KCH):
                        nc.tensor.matmul(ps1, w_sb[:, kk, 128:DH], x_t[:, kk, hc],
                                         start=(kk == 0), stop=(kk == KCH - 1),
                                         skip_group_check=True)
                    nc.vector.tensor_scalar_add(dst[0][c][0:64, dc], ps0[0:64, :], blo[0:64])
                    nc.sync.dma_start(out=dst[0][c][64:128, dc], in_=dst[0][c][0:64, dc])
                    nc.vector.tensor_scalar_add(dst[1][c][64:128, dc], ps0[64:128, :], blo[64:128])
                    nc.sync.dma_start(out=dst[1][c][0:64, dc], in_=dst[1][c][64:128, dc])
                    nc.vector.tensor_scalar_add(dst[2][c][0:64, dc], ps1[0:64, :], bhi[0:64])
                    nc.sync.dma_start(out=dst[2][c][64:128, dc], in_=dst[2][c][0:64, dc])

                def kv_group_pieces(p):
                    # fine-grained projection pieces for k/v chunk pair
                    # (2p, 2p+1), to be spread across inner-loop units
                    chunks = [c for c in (2 * p, 2 * p + 1) if c < NQ]
                    w = 512 * len(chunks)
                    ncols = bass.ds(chunks[0] * 512, w)
                    st = {}
                    k_eng = nc.gpsimd if p == 0 else nc.sync

                    def k_dma():
                        xk_t = xin.tile([128, KCH, w], BF16, tag="x_t",
                                        name="xk_t")
                        k_eng.dma_start(
                            out=xk_t,
                            in_=xkT.rearrange("(cc p) s -> p cc s",
                                              p=128)[:, :, ncols])
                        st["k"] = xk_t

                    def v_dma():
                        xv_t = xin.tile([128, KCH, w], BF16, tag="x_t",
                                        name="xv_t")
                        nc.scalar.dma_start(
                            out=xv_t,
                            in_=xvT.rearrange("(cc p) s -> p cc s",
                                              p=128)[:, :, ncols])
                        st["v"] = xv_t

                    def k_half(ci, half):
                        def f():
                            qk_halfproj(wk_sb, st["k"], kTr, bk_lo, bk_hi,
                                        chunks[ci], 2 * ci + half)
                        return f

                    def v_chunk(ci):
                        def f():
                            for sub in range(4):
                                vps = ps_proj.tile([128, 512], F32, tag="proj",
                                                   name="vps")
                                for kk in range(KCH):
                                    nc.tensor.matmul(
                                        vps[:, 0:DH],
                                        st["v"][:, kk,
                                                bass.ds(ci * 512 + sub * 128,
                                                        128)],
                                        wv_sb[:, kk, :],
                                        start=(kk == 0), stop=(kk == KCH - 1),
                                        skip_group_check=True,
                                    )
                                nc.vector.tensor_copy(
                                    v_c[chunks[ci]][:, sub, :, 0:64],
                                    vps[:, 0:DH].rearrange("p (h d) -> p h d",
                                                           h=3),
                                )
                        return f

                    out = [k_dma, v_dma, k_half(0, 0), k_half(0, 1)]
                    if len(chunks) > 1:
                        out += [k_half(1, 0), k_half(1, 1)]
                    out.append(v_chunk(0))
                    if len(chunks) > 1:
                        out.append(v_chunk(1))
                    return out

                def qproj_pieces(c):
                    st = {}

                    def p0():
                        xq_t = xin.tile([128, KCH, 512], BF16, tag="x_t",
                                        name="xq_t")
                        nc.sync.dma_start(
                            out=xq_t,
                            in_=xqT.rearrange("(cc p) s -> p cc s",
                                              p=128)[:, :, bass.ts(c, 512)])
                        st["q"] = xq_t
                        qk_halfproj(wq_sb, st["q"], qTr, bq_lo, bq_hi, c, 0)

                    def p1():
                        qk_halfproj(wq_sb, st["q"], qTr, bq_lo, bq_hi, c, 1)

                    return [p0, p1]

                def scores_pair(h, t2, qc):
                    t0, t1 = 2 * t2, 2 * t2 + 1
                    sc = ps_big.tile([128, 1024], F32, tag="big", name="sc")
                    nc.tensor.matmul(
                        sc[:, 0:512],
                        kTr[h][t0 // 4][0:64, bass.ts(t0 % 4, 128)],
                        qTr[h][qc][0:64, :],
                        start=True, stop=True, tile_position=(0, 0),
                    )
                    nc.tensor.matmul(
                        sc[:, 512:1024],
                        kTr[h][t1 // 4][64:128, bass.ts(t1 % 4, 128)],
                        qTr[h][qc][64:128, :],
                        start=True, stop=True, tile_position=(64, 0),
                    )
                    return sc

                def epilogue_job(qc, aT, j):
                    def f():
                        st = qc * 4 + j
                        jc = bass.ts(j, 128)
                        epi = ps_epi1.tile([128, 195], F32, tag="epi1", name="epi")
                        comb = epi_sb.tile([128, DH], BF16, tag="comb", name="comb")
                        for h in range(3):
                            nc.tensor.transpose(
                                epi[:, h * 65 : h * 65 + 65],
                                aT[h][:, jc],
                                id_f32[0:65, 0:65],
                            )
                        epi3 = epi.rearrange("p (h x) -> p h x", h=3)
                        rec = small.tile([128, 3], F32, tag="rec", name="rec")
                        nc.vector.reciprocal(rec, epi3[:, :, 64:65])
                        nc.vector.tensor_tensor(
                            out=comb.rearrange("p (h d) -> p h d", h=3),
                            in0=epi3[:, :, 0:64],
                            in1=rec[:, :, None].to_broadcast([128, 3, 64]),
                            op=mybir.AluOpType.mult,
                        )
                        nc.sync.dma_start(
                            out=comb_out[st * 128 : (st + 1) * 128, :],
                            in_=comb,
                        )
                    return f

                NPAIR = NT // 2
                units = [(qc, h, t2) for qc in range(NQ) for h in range(3)
                         for t2 in range(NPAIR)]
                NU = len(units)
                sc_q = {}

                def emit_scores_for(i):
                    qc_, h_, t2_ = units[i]
                    sc_q[i] = scores_pair(h_, t2_, qc_)

                # deadline-scheduled side work: unit index -> [thunks], run at
                # the top of that unit (before its scores emission)
                sched = {}

                def sched_add(i, thunk):
                    sched.setdefault(i, []).append(thunk)

                # k/v projections for chunk pairs (2p, 2p+1), p = 1..:
                # dmas issue right away (xin bufs cover the lookahead);
                # compute pieces land just ahead of the scores/attn that
                # need them
                for p in range(1, (NQ + 1) // 2):
                    pcs = kv_group_pieces(p)
                    base = 4 * (p - 1)
                    offs = (2 * (p - 1), 2 * (p - 1) + 1,  # kdma, vdma
                            base + 1, base + 2,            # k chunk 2p halves
                            base + 3, base + 4,            # k chunk 2p+1 halves
                            base + 3, base + 5)            # v chunks
                    for off, th in zip(offs, pcs):
                        sched_add(off, th)

                # startup: q chunk 0 + k/v chunks 0,1; first scores go as
                # soon as q proj + k chunk 0 are emitted
                g0 = kv_group_pieces(0)
                qp0 = qproj_pieces(0)
                qp0[0]()
                g0[0]()   # k dma
                qp0[1]()
                g0[2]()   # k chunk 0 half 0
                emit_scores_for(0)
                g0[3]()   # k chunk 0 half 1
                emit_scores_for(1)
                g0[1]()   # v dma
                g0[6]()   # v chunk 0
                g0[4]()   # k chunk 1 half 0
                g0[5]()   # k chunk 1 half 1
                g0[7]()   # v chunk 1

                aT = None
                accs = {}
                for i, (qc, h, t2) in enumerate(units):
                    for th in sched.pop(i, ()):
                        th()
                    if h == 0 and t2 == 0:
                        if qc + 1 < NQ:
                            qp = qproj_pieces(qc + 1)
                            qp[0]()
                            sched_add(i + 16, qp[1])
                        aT = [epi_sb.tile([65, 512], F32, tag=f"aT{hh}",
                                          name=f"aT{hh}") for hh in range(3)]
                    if t2 == 0:
                        accs[h] = ps_acc.tile([65, 512], F32, tag="acc",
                                              name="acc")
                    acc = accs[h]
                    sc_cur = sc_q.pop(i)
                    if t2 in DVE_PAIRS:
                        pri = probs_pool.tile([128, 1024], I16, tag="pri")
                        nc.vector.tensor_scalar(
                            out=pri, in0=sc_cur,
                            scalar1=SCH_A, scalar2=SCH_B,
                            op0=mybir.AluOpType.mult,
                            op1=mybir.AluOpType.add,
                        )
                        pr = pri.bitcast(BF16)
                    else:
                        pr = probs_pool.tile([128, 1024], BF16, tag="pr")
                        nc.scalar.activation(
                            out=pr, in_=sc_cur,
                            func=mybir.ActivationFunctionType.Exp,
                        )
                    if i + 2 < NU:
                        emit_scores_for(i + 2)
                    t0, t1 = 2 * t2, 2 * t2 + 1
                    nc.tensor.matmul(
                        acc, v_c[t0 // 4][:, t0 % 4, h, :], pr[:, 0:512],
                        start=(t2 == 0), stop=False,
                        skip_group_check=True,
                    )
                    nc.tensor.matmul(
                        acc, v_c[t1 // 4][:, t1 % 4, h, :], pr[:, 512:1024],
                        start=False, stop=(t2 == NPAIR - 1),
                        skip_group_check=True,
                    )
                    if t2 == NPAIR - 1:
                        nc.scalar.copy(aT[h], acc)
                        if h == 2:
                            for j in range(4):
                                sched_add(i + 2 + 2 * j, epilogue_job(qc, aT, j))
                for i in sorted(sched):
                    for th in sched[i]:
                        th()

    if split:
        split_multi_waits(nc)
    return nc


_NC_CACHE = {}


def _get_nc(S):
    if S not in _NC_CACHE:
        _NC_CACHE[S] = build_nc(S)
    return _NC_CACHE[S]


def shard_inputs(Q, K, V, Wq, bq, Wk, bk, Wv, bv, Wo, bo, S):
    """Build the 8 per-core input maps (numpy, host-side shard+cast)."""
    in_maps = []
    for c in range(8):
        b = c // 4
        r0 = 3 * (c % 4) * 64
        rows = slice(r0, r0 + DH)
        in_maps.append(
            {
                "xqT": np.ascontiguousarray(Q[b].T).astype(_bf16np),
                "xkT": np.ascontiguousarray(K[b].T).astype(_bf16np),
                "xvT": np.ascontiguousarray(V[b].T).astype(_bf16np),
                "wqT": np.ascontiguousarray(Wq[rows].T / 8.0).astype(_bf16np),
                "wkT": np.ascontiguousarray(Wk[rows].T).astype(_bf16np),
                "wvT": np.ascontiguousarray(Wv[rows].T).astype(_bf16np),
                "bq": (bq[rows] / 8.0).reshape(DH, 1).astype(np.float32),
                "bk": bk[rows].reshape(DH, 1).astype(np.float32),
            }
        )
    return in_maps


def gather_output(results, Q, bv, Wo, bo):
    B, S = Q.shape[0], Q.shape[1]
    out = np.zeros((B, S, D_MODEL), np.float32)
    WoT = Wo.T.astype(np.float32)  # [768 in-dims, 768 out]
    for c, r in enumerate(results):
        r0 = 3 * (c % 4) * 64
        out[c // 4] += r["comb"].astype(np.float32) @ WoT[r0 : r0 + DH, :]
    out += (bv.astype(np.float32) @ WoT + bo.astype(np.float32))[None, None, :]
    return out


def kernel(Q, K, V, Wq, bq, Wk, bk, Wv, bv, Wo, bo, **run_kwargs):
    Q, K, V, Wq, bq, Wk, bk, Wv, bv, Wo, bo = (
        np.asarray(a) for a in (Q, K, V, Wq, bq, Wk, bk, Wv, bv, Wo, bo)
    )
    S = Q.shape[1]
    nc = _get_nc(S)
    in_maps = shard_inputs(Q, K, V, Wq, bq, Wk, bk, Wv, bv, Wo, bo, S)
    res = run_bass_kernel_spmd(nc, in_maps, core_ids=list(range(8)), **run_kwargs)
    out = gather_output(res.results, Q, bv, Wo, bo)
    kernel.last_results = res
    return out

